# revision 1
# baseline (speedup 1.0000x reference)
"""BitLinear (BitNet b1.58-style) forward on 8 Trainium2 NeuronCores.

Column-parallel sharding: weight rows (out_features) split across 8
cores, activations replicated, outputs concatenated on the host.

Math (per core, exact-integer formulation):
  X_int = clip(round(X * 128/max(rowmax|X|, EPS)), -128, 127)  [per-token]
  W_int = clip(round(W / w_scale), -1, 1)                       [ternary]
  Y     = (X_int @ W_int^T) * (rowmax_c/128) * w_scale

X_int in [-128,127] and W_int in {-1,0,1} are exact in bf16; products
accumulate exactly in fp32 PSUM (|sum| < 2^21 << 2^24), so the bf16
tensor-engine matmul computes bit-exact integer arithmetic at full
bf16 rate.

Rounding uses the f32 magic-number trick: (t + 1.5*2^23) - 1.5*2^23 ==
rint(t) (round-half-even, matching jnp.round) for |t| <= 2^22.
"""

import sys

try:
    import concourse.bass as bass  # noqa: F401
except ImportError:
    sys.path.insert(0, "/opt/trn_rl_repo")

import numpy as np

import concourse.tile as tile
from concourse import bacc, mybir
from concourse.bass_utils import run_bass_kernel_spmd
from concourse.masks import make_identity

F32 = mybir.dt.float32
BF16 = mybir.dt.bfloat16
FP8 = mybir.dt.float8e4

MAGIC = 12582912.0  # 1.5 * 2**23
EPS = 1e-5
P = 128

N_CORES = 8
B, S, K_IN, N_OUT = 4, 2048, 4096, 16384
M_FULL = B * S
N_SHARD = N_OUT // N_CORES


def build_body(tc, x_ap, w_ap, s_ap, y_ap, xpose="dma"):
    nc = tc.nc
    M, K = x_ap.shape
    K2, N = w_ap.shape  # w arrives host-transposed: [K, N]
    assert K == K2
    MT, KT, NT = M // P, K // P, N // P
    NCH = min(512, N)  # psum chunk (free dim per matmul)
    NC = N // NCH

    G = 4  # m-tiles per group (chunk-major within a group)
    import contextlib

    ctx = contextlib.ExitStack()
    with ctx:
        const = ctx.enter_context(tc.tile_pool(name="const", bufs=1))
        inrows = ctx.enter_context(tc.tile_pool(name="inrows", bufs=3))
        wslabp = ctx.enter_context(tc.tile_pool(name="wslab", bufs=3))
        xqp = ctx.enter_context(tc.tile_pool(name="xq", bufs=3))
        xtp = ctx.enter_context(tc.tile_pool(name="xt", bufs=G + 2))
        wtallp = ctx.enter_context(tc.tile_pool(name="wtall", bufs=1))
        yp = ctx.enter_context(tc.tile_pool(name="y", bufs=4))
        rvp = ctx.enter_context(tc.tile_pool(name="rv", bufs=32))
        psy = ctx.enter_context(tc.tile_pool(name="psy", bufs=6, space="PSUM"))
        pst = (
            ctx.enter_context(tc.tile_pool(name="pst", bufs=2, space="PSUM"))
            if xpose != "dma" else None
        )

        ident = const.tile([P, P], BF16)
        make_identity(nc, ident[:])
        svec = const.tile([P, 2], F32)
        nc.sync.dma_start(svec[:], s_ap)
        winv = svec[:, 0:1]  # 1/w_scale
        wdiv = svec[:, 1:2]  # w_scale/128
        mgap = const.tile([P, 1], F32)
        nc.vector.memset(mgap[:], MAGIC)
        nmgap = const.tile([P, 1], F32)
        nc.vector.memset(nmgap[:], -MAGIC)

        wtall = wtallp.tile([P, KT, N], FP8)

        # ---- W prep: w arrives host-transposed [K, N]; quantize to
        # ternary fp8 slab-by-slab straight into the [k, n] matmul layout.
        # n-chunk-major so the first 512 output columns unblock matmuls
        # early; multiply+round on ACT, clamp on DVE.
        for jc in range(NC):
            for kk in range(KT):
                wslab = wslabp.tile([P, NCH], F32, tag="wslab")
                nc.sync.dma_start(
                    wslab[:],
                    w_ap[kk * P : (kk + 1) * P, jc * NCH : (jc + 1) * NCH],
                )
                # u = w * (1/ws) + MAGIC   (the f32 add performs rint)
                nc.scalar.activation(
                    wslab[:], wslab[:], mybir.ActivationFunctionType.Identity,
                    bias=mgap[:], scale=winv,
                )
                # clip to [MAGIC-1, MAGIC+1]
                nc.vector.tensor_scalar(
                    wslab[:], wslab[:], MAGIC - 1.0, MAGIC + 1.0,
                    op0=mybir.AluOpType.max, op1=mybir.AluOpType.min,
                )
                # wq = u - MAGIC -> fp8 (exact ternary); alternate the
                # engine per slab to balance DVE/ACT during startup
                wdst = wtall[:, kk, jc * NCH : (jc + 1) * NCH]
                if kk % 2 == 0:
                    nc.vector.tensor_scalar(
                        wdst, wslab[:], MAGIC, None,
                        op0=mybir.AluOpType.subtract,
                    )
                else:
                    nc.scalar.activation(
                        wdst, wslab[:],
                        mybir.ActivationFunctionType.Identity, bias=nmgap[:],
                    )

        # ---- main loop over groups of token tiles ----
        for g in range(0, MT, G):
            oscs = []
            xts = []
            for m in range(g, min(g + G, MT)):
                xrow = inrows.tile([P, K], F32, tag="inrow")
                nc.sync.dma_start(xrow[:], x_ap[m * P : (m + 1) * P, :])

                rmax = rvp.tile([P, 1], F32, tag="rv")
                nc.vector.tensor_reduce(
                    rmax[:], xrow[:], axis=mybir.AxisListType.X,
                    op=mybir.AluOpType.max, apply_absolute_value=True,
                )
                rmaxc = rvp.tile([P, 1], F32, tag="rv")
                nc.vector.tensor_scalar_max(rmaxc[:], rmax[:], EPS)
                rinv = rvp.tile([P, 1], F32, tag="rv")
                nc.vector.reciprocal(rinv[:], rmaxc[:])
                s128 = rvp.tile([P, 1], F32, tag="rv")
                nc.vector.tensor_scalar_mul(s128[:], rinv[:], 128.0)
                osc = rvp.tile([P, 1], F32, tag="rv")
                nc.vector.tensor_scalar(
                    osc[:], rmaxc[:], wdiv, None, op0=mybir.AluOpType.mult,
                )
                oscs.append(osc)

                # v = min(x * s128, 126.9)  (in-place over xrow)
                nc.vector.tensor_scalar(
                    xrow[:], xrow[:], s128[:], 126.9,
                    op0=mybir.AluOpType.mult, op1=mybir.AluOpType.min,
                )
                # xq = ((v + MAGIC) - MAGIC) -> bf16  (exact int8-valued)
                xq = xqp.tile([P, K], BF16, tag="xq")
                nc.vector.tensor_scalar(
                    xq[:], xrow[:], MAGIC, MAGIC, op0=mybir.AluOpType.add,
                    op1=mybir.AluOpType.subtract,
                )

                # transpose xq -> xt [k, m]
                xt = xtp.tile([P, KT, P], BF16)
                if xpose == "dma":
                    nc.sync.dma_start_transpose(xt[:], xq[:])
                else:
                    for kk in range(KT):
                        ps = pst.tile([P, P], BF16)
                        nc.tensor.transpose(
                            ps[:], xq[:, kk * P : (kk + 1) * P], ident[:]
                        )
                        nc.scalar.copy(xt[:, kk, :], ps[:])
                xts.append(xt)

            # chunk-major matmuls: W chunk j is consumed by G chains
            # before chunk j+1 is needed.
            for j in range(NC):
                for gi, m in enumerate(range(g, min(g + G, MT))):
                    ps = psy.tile([P, NCH], F32)
                    for kk in range(KT):
                        nc.tensor.matmul(
                            ps[:], xts[gi][:, kk, :],
                            wtall[:, kk, j * NCH : (j + 1) * NCH],
                            start=(kk == 0), stop=(kk == KT - 1),
                        )
                    ysb = yp.tile([P, NCH], F32, tag="y")
                    nc.scalar.mul(ysb[:], ps[:], oscs[gi][:])
                    nc.sync.dma_start(
                        y_ap[m * P : (m + 1) * P, j * NCH : (j + 1) * NCH],
                        ysb[:],
                    )


def build_program(M, K, N, n_cores=N_CORES, repeat=1, debug=False, xpose="dma"):
    nc = bacc.Bacc(
        "TRN2", target_bir_lowering=False, debug=debug, num_devices=n_cores
    )
    x_ap = nc.dram_tensor("x", [M, K], F32, kind="ExternalInput").ap()
    w_ap = nc.dram_tensor("w", [K, N], F32, kind="ExternalInput").ap()
    s_ap = nc.dram_tensor("s", [P, 2], F32, kind="ExternalInput").ap()
    y_ap = nc.dram_tensor("y", [M, N], F32, kind="ExternalOutput").ap()
    with tile.TileContext(nc) as tc:
        if repeat == 1:
            build_body(tc, x_ap, w_ap, s_ap, y_ap, xpose=xpose)
        else:
            with tc.For_i(0, repeat, 1) as _i:
                build_body(tc, x_ap, w_ap, s_ap, y_ap, xpose=xpose)
    nc.compile()
    return nc


def make_inputs_np(x_full, weight_full, n_cores=N_CORES):
    """Host-side sharding: returns (in_maps, w_scale)."""
    k_in = x_full.shape[-1]
    xm = np.ascontiguousarray(x_full.reshape(-1, k_in), dtype=np.float32)
    w_scale = float(
        max(np.mean(np.abs(weight_full), dtype=np.float32), np.float32(EPS))
    )
    svec = np.zeros((P, 2), np.float32)
    svec[:, 0] = np.float32(1.0) / np.float32(w_scale)
    svec[:, 1] = np.float32(w_scale) / np.float32(128.0)
    nshard = weight_full.shape[0] // n_cores
    in_maps = [
        {
            "x": xm,
            "w": np.ascontiguousarray(
                weight_full[c * nshard : (c + 1) * nshard].T, dtype=np.float32
            ),
            "s": svec,
        }
        for c in range(n_cores)
    ]
    return in_maps, w_scale


_NC_CACHE = {}
DEFAULT_XPOSE = "dma"


def _get_program():
    key = (M_FULL, K_IN, N_SHARD, N_CORES, DEFAULT_XPOSE)
    if key not in _NC_CACHE:
        _NC_CACHE[key] = build_program(
            M_FULL, K_IN, N_SHARD, N_CORES, xpose=DEFAULT_XPOSE
        )
    return _NC_CACHE[key]


def kernel(x, weight):
    x = np.asarray(x)
    weight = np.asarray(weight)
    assert x.shape == (B, S, K_IN) and weight.shape == (N_OUT, K_IN)
    nc = _get_program()
    in_maps, _ = make_inputs_np(x, weight, N_CORES)
    res = run_bass_kernel_spmd(nc, in_maps, list(range(N_CORES)))
    y = np.concatenate(
        [res.results[c]["y"] for c in range(N_CORES)], axis=1
    )
    return np.ascontiguousarray(y.reshape(B, S, N_OUT), dtype=np.float32)

